# revision 18
# baseline (speedup 1.0000x reference)
"""Causal self-attention with RoPE + XSA (self-value subtraction), Trainium2.

Sharding: (batch x head-half). 8 cores = 4 batches x 2 groups of 8 heads.
Each core: QKV for its batch/head-group, flash-style causal attention in
S^T layout (k on partitions, q on free dim), partial output projection over
its 512 feature columns. Host sums 2 partials per batch.

v12, final (from the 455us v2 baseline; HW-verified 316.1us max core,
313.9us mean, rel err 5.97e-3):
- S^T matmuls (K=64, half the PE array) issued as head PAIRS on PE
  row-tile positions (0,0)/(64,0): both stream CONCURRENTLY (measured
  start delta 3ns), halving attention-score cost.
- exp is one ACT instr per pair over a [128,2,512] 2-bank PSUM tile
  (ACT has ~300-440ns fixed cost per instr, so fusing matters).
- rope runs entirely on DVE as per-ti single-free-dim ops (multi-dim
  free APs fall off the 2x DVE fast path: 1226ns vs 410ns); gpsimd is
  reserved for partition_broadcast (its tensor_tensor is 1.3us/op and
  microcode lib thrash stalled the PE).
- PSUM plan (8 banks): pst pair pool 2x2 + po pair [65,2,512] 2 +
  qkv/proj/diag pool 2. po is evacuated to SBUF right after the last
  V matmul (ACT for qj<3, DVE at qj=3 where ACT is exp-saturated;
  skipped entirely for the final pair to shorten the tail chain).
- per-pair epilogue: 2 den adds (read po PSUM directly: PSUM+SB
  operands are exempt from the SB+SB equal-base-partition rule), one
  reciprocal + one gpsimd partition_broadcast (split per head for the
  final pair), deferred into the next pair's kc loop.
- PSUM->SBUF copies split between ACT and DVE by phase so neither
  queue backs up; proj fillers deferred toward qj=3 (exp-bound there,
  so extra PE work hides the pst-reuse stalls).
- tail proj streams fi=0..2 for both nj while the last epilogue
  drains; only the fi=3 matmuls wait on it. DMAs per-nj on rotating
  queues.
- prologue: per-fc 128KB weight/x DMAs (big strided DMAs ran at 1/3
  bandwidth); cos/sin first-512-token chunks load before the rest.

Layout notes (per core):
  A_q, A_k : [128, 4, 2048] bf16  q^T/k^T; tile p rows 0..63 = head 2p,
             64..127 = head 2p+1 (dims within head).
  v_sb     : [128, 16, 8, 65] bf16  v token-major per 128-tok chunk per
             head + ones col (col 64) for the softmax denominator row.
  attention: S^T[k, q] = matmul(lhsT=K^T[d, kc*128:], rhs=Q^T[d, qj*512:])
             per head pair on PE row tiles; P = exp(S^T/8) -> bf16 in one
             [128,2,512] instr; V-matmuls give out^T[d(+denom), q].
  XSA      : strict mask (k<q) zeroes diag+future; diag exp added to the
             denominator via a tiny K=2 matmul from elementwise q.k products.
"""

import sys

if "/opt/trn_rl_repo" not in sys.path:
    sys.path.insert(0, "/opt/trn_rl_repo")

import numpy as np

B, T, D, H = 4, 2048, 1024, 16
DH = D // H  # 64
HALF = DH // 2  # 32
NCORES = 8
HPC = 8  # heads per core
QC = 512  # q chunk
KC = 128  # k chunk
NQJ = T // QC  # 4 q chunks
FC = D // 128  # 8 input-feature chunks


def _build():
    import concourse.bass as bass
    import concourse.mybir as mybir
    import concourse.tile as tile
    from concourse import bacc

    F32 = mybir.dt.float32
    BF16 = mybir.dt.bfloat16
    AF = mybir.ActivationFunctionType
    ALU = mybir.AluOpType
    ds, ts = bass.ds, bass.ts

    nc = bacc.Bacc("TRN2")

    xT_d = nc.dram_tensor("xT", (D, T), BF16, kind="ExternalInput")
    wq_d = nc.dram_tensor("wqT", (D, 512), BF16, kind="ExternalInput")
    wk_d = nc.dram_tensor("wkT", (D, 512), BF16, kind="ExternalInput")
    wv_d = nc.dram_tensor("wvT", (D, 512), BF16, kind="ExternalInput")
    wp_d = nc.dram_tensor("wpT", (512, D), BF16, kind="ExternalInput")
    cosr_d = nc.dram_tensor("cosr", (128, T), BF16, kind="ExternalInput")
    sinr_d = nc.dram_tensor("sinr", (128, T), BF16, kind="ExternalInput")
    esel_d = nc.dram_tensor("esel", (128, 33), BF16, kind="ExternalInput")
    strictu_d = nc.dram_tensor("strictu", (128, 2, 128), BF16,
                               kind="ExternalInput")
    out_d = nc.dram_tensor("outp", (T, D), F32, kind="ExternalOutput")

    with tile.TileContext(nc) as tc:
        with (
            tc.tile_pool(name="p1", bufs=1) as p1,
            tc.tile_pool(name="pxt", bufs=2) as pxt,
            tc.tile_pool(name="ppt", bufs=5) as ppt,
            tc.tile_pool(name="pbt", bufs=2) as pbt,
            tc.tile_pool(name="psc", bufs=2) as psc,
            tc.tile_pool(name="ps2", bufs=2, space="PSUM") as ps2,
            tc.tile_pool(name="psm", bufs=2, space="PSUM") as psm,
            tc.tile_pool(name="pso", bufs=1, space="PSUM") as pso,
        ):
            # --- persistent weights / constants ---
            # weights on the ACT HWDGE queue (wq/wk split per feature chunk
            # so QKV matmuls start as soon as (xt[0], wq[0]) land); small
            # tables + 2x-replicated rope tables on the gpsimd queue so
            # neither blocks the other.
            xt0 = pxt.tile([128, FC, QC], BF16, tag="xt", name="xt0")
            for fc in range(FC):
                nc.sync.dma_start(xt0[:, fc, :], xT_d[ts(fc, 128), 0:QC])
            wq_sb = p1.tile([128, FC, 512], BF16, tag="wq")
            wq_r = wq_d[:].rearrange("(o p) m -> p o m", p=128)
            for fc in range(FC):
                nc.scalar.dma_start(wq_sb[:, fc], wq_r[:, fc])
            wk_sb = p1.tile([128, FC, 512], BF16, tag="wk")
            wk_r = wk_d[:].rearrange("(o p) m -> p o m", p=128)
            for fc in range(FC):
                nc.scalar.dma_start(wk_sb[:, fc], wk_r[:, fc])
            wv_sb = p1.tile([128, FC, 512], BF16, tag="wv")
            wv_r = wv_d[:].rearrange("(o p) m -> p o m", p=128)
            for fc in range(FC):
                nc.sync.dma_start(wv_sb[:, fc], wv_r[:, fc])

            esel_sb = p1.tile([128, 33], BF16, tag="esel")
            nc.gpsimd.dma_start(esel_sb[:], esel_d[:])
            strictu2 = p1.tile([128, 2, 128], BF16, tag="strictu")
            nc.gpsimd.dma_start(strictu2[:], strictu_d[:])
            cosr = p1.tile([128, T], BF16, tag="cosr")
            sinr = p1.tile([128, T], BF16, tag="sinr")
            for c in range(NQJ):
                nc.gpsimd.dma_start(cosr[:, ts(c, QC)], cosr_d[:, ts(c, QC)])
                nc.gpsimd.dma_start(sinr[:, ts(c, QC)], sinr_d[:, ts(c, QC)])
            wp_sb = p1.tile([128, 4, D], BF16, tag="wp")
            nc.scalar.dma_start(
                wp_sb[:], wp_d[:].rearrange("(o p) m -> p o m", p=128))

            A_q = p1.tile([128, 4, T], BF16, tag="A_q")
            A_k = p1.tile([128, 4, T], BF16, tag="A_k")
            qkp = p1.tile([128, 4, T], BF16, tag="qkp")
            v_sb = p1.tile([128, T // 128, HPC, 65], BF16, tag="v_sb")
            outT = p1.tile([128, 4, T], BF16, tag="outT")
            # ones column for the denominator row of the V-matmul
            nc.gpsimd.memset(v_sb[:, :, :, 64], 1.0)

            def rope_g(A, tcs, g):
                # RoPE on ti-group g: A = A*cos + swap(A)*sin (signs baked
                # into sinr). swap via 4 SBUF DMAs; per-ti single-free-dim
                # DVE ops (multi-dim free APs fall off the 2x DVE fast path)
                gg = ds(2 * g, 2)
                Bt = pbt.tile([128, 2, QC], BF16, tag="Bt")
                nc.sync.dma_start(Bt[0:32], A[32:64, gg, tcs])
                nc.sync.dma_start(Bt[32:64], A[0:32, gg, tcs])
                nc.sync.dma_start(Bt[64:96], A[96:128, gg, tcs])
                nc.sync.dma_start(Bt[96:128], A[64:96, gg, tcs])
                for j in range(2):
                    ti = 2 * g + j
                    nc.vector.tensor_tensor(Bt[:, j, :], Bt[:, j, :],
                                            sinr[:, tcs], ALU.mult)
                    nc.vector.tensor_tensor(A[:, ti, tcs], A[:, ti, tcs],
                                            cosr[:, tcs], ALU.mult)
                    nc.vector.tensor_tensor(A[:, ti, tcs], A[:, ti, tcs],
                                            Bt[:, j, :], ALU.add)

            def qkv_units(tci):
                """Generator: emit QKV for token chunk tci in PE-sized units.

                Rope for a ti-pair is issued right after its 2 tiles land so
                attention(tci) never waits on a long rope chain; copies are
                DVE so the ACT engine stays free for exp."""
                tok0 = tci * QC
                tcs = ds(tok0, QC)
                if tci == 0:
                    xt = xt0
                else:
                    xt = pxt.tile([128, FC, QC], BF16, tag="xt",
                                  name=f"xt{tci}")
                    for fc in range(FC):
                        nc.sync.dma_start(xt[:, fc, :], xT_d[ts(fc, 128), tcs])
                for w_sb, dst in ((wq_sb, A_q), (wk_sb, A_k)):
                    for ti in range(4):
                        pq = psm.tile([128, QC], F32, tag="sm", name="pq")
                        for fc in range(FC):
                            nc.tensor.matmul(
                                pq[:],
                                w_sb[:, fc, ts(ti, 128)],
                                xt[:, fc, :],
                                start=(fc == 0),
                                stop=(fc == FC - 1),
                            )
                            if fc == 3:
                                yield
                        cp = (nc.scalar.copy if ti % 2
                              else nc.vector.tensor_copy)
                        cp(dst[:, ti, tcs], pq[:])
                        yield
                        if ti % 2 == 1:
                            rope_g(dst, tcs, ti // 2)
                for t4 in range(4):
                    pv = psm.tile([128, QC], F32, tag="sm", name="pv")
                    for fc in range(FC):
                        nc.tensor.matmul(
                            pv[:],
                            xt[:, fc, ts(t4, 128)],
                            wv_sb[:, fc, :],
                            start=(fc == 0),
                            stop=(fc == FC - 1),
                        )
                        if fc == 3:
                            yield
                    cp = (nc.scalar.copy if t4 % 2
                          else nc.vector.tensor_copy)
                    cp(
                        v_sb[:, 4 * tci + t4, :, 0:64],
                        pv[:].rearrange("p (h d) -> p h d", h=HPC),
                    )
                    yield
                # elementwise q.k products (diag exp source), post-rope
                for ti in range(4):
                    nc.vector.tensor_tensor(
                        qkp[:, ti, tcs], A_q[:, ti, tcs], A_k[:, ti, tcs],
                        ALU.mult,
                    )

            def proj_units(qj, tail=False):
                """Generator: project 512 tokens of q-chunk qj to DRAM."""
                for nt in range(4):
                    mt0 = qj * QC + nt * 128
                    po_s = psc.tile([128, 2, QC], F32, tag="po_sb", bufs=2)
                    pps = []
                    if tail:
                        # both nj accumulations open: fi=0..2 stream while the
                        # last pair's epilogue (which produces fi=3's outT)
                        # drains; only the fi=3 matmuls wait on it
                        for nj in range(2):
                            pp = psm.tile([128, QC], F32, tag="sm", name="pp")
                            pps.append(pp)
                            for fi in range(3):
                                nc.tensor.matmul(
                                    pp[:],
                                    outT[:, fi, ds(mt0, 128)],
                                    wp_sb[:, fi, ts(nj, 512)],
                                    start=(fi == 0),
                                    stop=False,
                                )
                        for nj in range(2):
                            nc.tensor.matmul(
                                pps[nj][:],
                                outT[:, 3, ds(mt0, 128)],
                                wp_sb[:, 3, ts(nj, 512)],
                                start=False,
                                stop=True,
                            )
                    for nj in range(2):
                        if tail:
                            pp = pps[nj]
                        else:
                            pp = psm.tile([128, QC], F32, tag="sm", name="pp")
                            for fi in range(4):
                                nc.tensor.matmul(
                                    pp[:],
                                    outT[:, fi, ds(mt0, 128)],
                                    wp_sb[:, fi, ts(nj, 512)],
                                    start=(fi == 0),
                                    stop=(fi == 3),
                                )
                        if tail:
                            # spread tail copies + DMAs so the post-compute
                            # drain is short
                            cp = (nc.scalar.copy, nc.vector.tensor_copy)[nj]
                            cp(po_s[:, nj], pp[:])
                            eng = (nc.sync, nc.gpsimd, nc.scalar, nc.sync)[
                                (2 * nt + nj) % 4
                            ]
                            eng.dma_start(
                                out_d[ds(mt0, 128), ts(nj, 512)],
                                po_s[:, nj],
                            )
                        else:
                            nc.vector.tensor_copy(po_s[:, nj], pp[:])
                        yield
                    if not tail:
                        nc.sync.dma_start(
                            out_d[ds(mt0, 128), :],
                            po_s[:].rearrange("p a b -> p (a b)"),
                        )
                    yield

            def emit_attention(qj, fillers, epi_in=None):
                """Pair-parallel causal attention for q-chunk qj.

                Each head pair's S^T matmuls go to PE row tiles (0,0)/(64,0)
                and execute concurrently; exp covers both heads in one ACT
                instr. The divide-and-store epilogue is deferred into the
                NEXT pair's kc loop so the PE never waits on the ACT round
                trip. Returns the last pair's epilogue closure."""
                q0 = qj * QC
                nkc = 4 * qj + 4
                epi = epi_in
                for p in range(4):
                    hA, hB = 2 * p, 2 * p + 1
                    last = qj == NQJ - 1 and p == 3
                    po = pso.tile([65, 2, QC], F32, tag="po", name="po")
                    pos_ = (None if last else
                            psc.tile([65, 2, QC], F32, tag="posb", bufs=2))
                    de = None
                    pending = None
                    for kc in range(nkc):
                        o = kc - 4 * qj
                        c0 = 128 * o if o > 0 else 0
                        cs = ds(q0 + c0, QC - c0)
                        pst = ps2.tile([128, 2, QC], F32, tag="pst",
                                       name="pst")
                        nc.tensor.matmul(
                            pst[:, 0, c0:QC],
                            A_k[0:64, p, ts(kc, 128)],
                            A_q[0:64, p, cs],
                            start=True,
                            stop=True,
                        )
                        nc.tensor.matmul(
                            pst[:, 1, c0:QC],
                            A_k[64:128, p, ts(kc, 128)],
                            A_q[64:128, p, cs],
                            start=True,
                            stop=True,
                        )
                        pt = ppt.tile([128, 2, QC], BF16, tag="pt", name="pt")
                        nc.scalar.activation(
                            pt[:, :, c0:QC], pst[:, :, c0:QC], AF.Exp,
                            scale=0.125,
                        )
                        if o >= 0:
                            nc.vector.tensor_tensor(
                                pt[:, :, ds(c0, 128)],
                                pt[:, :, ds(c0, 128)],
                                strictu2[:],
                                ALU.mult,
                            )
                        next(fillers, None)
                        if kc == 2:
                            # prefetch XSA diag term: de = exp(q.k/8), both
                            # heads as rows 0/1 of a K=2 matmul
                            pd_t = psm.tile([128, QC], F32, tag="sm",
                                            name="pd")
                            nc.tensor.matmul(
                                pd_t[0:33, :], esel_sb[:],
                                qkp[:, p, ds(q0, QC)],
                                start=True, stop=True,
                            )
                            de = psc.tile([33, QC], F32, tag="de", bufs=2)
                            nc.scalar.activation(
                                de[:], pd_t[0:33, :], AF.Exp, scale=0.125
                            )
                        if kc == 1 and epi is not None:
                            epi()
                            epi = None
                        if pending is not None:
                            pkc, ptt, pc0 = pending
                            nc.tensor.matmul(
                                po[:, 0, pc0:QC],
                                v_sb[:, pkc, hA, :],
                                ptt[:, 0, pc0:QC],
                                start=(pkc == 0),
                                stop=False,
                            )
                            nc.tensor.matmul(
                                po[:, 1, pc0:QC],
                                v_sb[:, pkc, hB, :],
                                ptt[:, 1, pc0:QC],
                                start=(pkc == 0),
                                stop=False,
                            )
                        pending = (kc, pt, c0)
                    pkc, ptt, pc0 = pending
                    nc.tensor.matmul(
                        po[:, 0, pc0:QC],
                        v_sb[:, pkc, hA, :],
                        ptt[:, 0, pc0:QC],
                        start=(pkc == 0),
                        stop=True,
                    )
                    nc.tensor.matmul(
                        po[:, 1, pc0:QC],
                        v_sb[:, pkc, hB, :],
                        ptt[:, 1, pc0:QC],
                        start=(pkc == 0),
                        stop=True,
                    )
                    # evacuate po to SBUF so its 2 banks free up fast
                    # (skipped for the final pair: nothing reuses po, and the
                    # tail is latency-critical). The den adds read po (PSUM)
                    # inline — PSUM+SBUF operands are exempt from the
                    # equal-base-partition rule that SB+SB tensor_tensor
                    # must obey.
                    if not last:
                        # ACT while it has surplus (qj<3); DVE at qj=3 where
                        # ACT is exp-saturated. On DVE in qj<3 this 1.2us
                        # copy stacked with rope/epilogue ops and stalled
                        # QKV copies at pair boundaries.
                        cp = (nc.vector.tensor_copy if qj == NQJ - 1
                              else nc.scalar.copy)
                        cp(pos_[:], po[:])
                    dnr = psc.tile([1, 2, QC], F32, tag="dnr", bufs=2)
                    nc.vector.tensor_tensor(
                        dnr[:, 0], po[64:65, 0], de[0:1], ALU.add
                    )
                    nc.vector.tensor_tensor(
                        dnr[:, 1], po[64:65, 1], de[32:33], ALU.add
                    )
                    if epi is not None:
                        epi()
                        epi = None

                    def mk_epi(pos_=(po if last else pos_), dnr=dnr,
                               p=p, last=last):
                        def run():
                            # one reciprocal + one partition broadcast per
                            # pair; for the final pair split per head so the
                            # tail's serial chain (rec->bc->mult) is shorter
                            rec = psc.tile([1, 2, QC], F32, tag="rec",
                                           bufs=1)
                            bc = psc.tile([64, 2, QC], F32, tag="bc", bufs=1)
                            if last:
                                for g in range(2):
                                    nc.vector.reciprocal_approx_fast(
                                        rec[:, g], dnr[:, g])
                                    nc.gpsimd.partition_broadcast(
                                        bc[:, g], rec[:, g])
                            else:
                                nc.vector.reciprocal_approx_fast(
                                    rec[:], dnr[:])
                                nc.gpsimd.partition_broadcast(bc[:], rec[:])
                            nc.vector.tensor_tensor(
                                outT[0:64, p, ds(q0, QC)],
                                pos_[0:64, 0],
                                bc[:, 0],
                                ALU.mult,
                            )
                            nc.vector.tensor_tensor(
                                outT[64:128, p, ds(q0, QC)],
                                pos_[0:64, 1],
                                bc[:, 1],
                                ALU.mult,
                            )
                        return run

                    epi = mk_epi()
                return epi

            def chain_rr(gens):
                active = list(gens)
                while active:
                    keep = []
                    for g in active:
                        try:
                            next(g)
                            keep.append(g)
                            yield
                        except StopIteration:
                            pass
                    active = keep

            # prologue: QKV for token chunk 0
            for _ in qkv_units(0):
                pass
            epi = None
            # proj fillers deferred toward qj=3, which is otherwise
            # ACT(exp)-bound with little PE work to hide the stalls
            proj_sched = {2: [0], 3: [1, 2]}
            for qj in range(NQJ):
                gens = []
                if qj + 1 < NQJ:
                    gens.append(qkv_units(qj + 1))
                for pj in proj_sched.get(qj, []):
                    gens.append(proj_units(pj))
                fillers = chain_rr(gens)
                epi = emit_attention(qj, fillers, epi_in=epi)
                epi()  # last pair's epilogue: outT[:, :, qj] complete
                epi = None
                for _ in fillers:
                    pass
            for _ in proj_units(NQJ - 1, tail=True):
                pass

    nc.finalize()
    return nc


def _host_inputs(x, cos, sin, W_qkv, W_proj):
    """Build per-core input maps. Core c = batch (c//2), head-group (c%2)."""
    import ml_dtypes

    bf16 = ml_dtypes.bfloat16
    x = np.asarray(x, dtype=np.float32)
    cos = np.asarray(cos, dtype=np.float32)
    sin = np.asarray(sin, dtype=np.float32)
    W_qkv = np.asarray(W_qkv, dtype=np.float32)
    W_proj = np.asarray(W_proj, dtype=np.float32)

    cosT = np.ascontiguousarray(cos[0, 0].T)  # [32, T]
    sinT = np.ascontiguousarray(sin[0, 0].T)
    cosr = np.tile(cosT, (4, 1)).astype(bf16)  # [128, T]
    sinr = np.concatenate([-sinT, sinT, -sinT, sinT], axis=0).astype(bf16)

    esel = np.zeros((128, 33), np.float32)
    esel[0:64, 0] = 1.0
    esel[64:128, 32] = 1.0
    esel = esel.astype(bf16)
    s1 = np.triu(np.ones((128, 128), np.float32), 1)
    strictu = np.stack([s1, s1], axis=1).astype(bf16)  # [128, 2, 128]

    # per head-group weight slices
    wslices = []
    for g in range(2):
        hs = range(8 * g, 8 * g + 8)
        wq = np.concatenate([W_qkv[64 * h : 64 * h + 64] for h in hs], axis=0)
        wk = np.concatenate(
            [W_qkv[D + 64 * h : D + 64 * h + 64] for h in hs], axis=0
        )
        wv = np.concatenate(
            [W_qkv[2 * D + 64 * h : 2 * D + 64 * h + 64] for h in hs], axis=0
        )
        wqT = np.ascontiguousarray(wq.T).astype(bf16)  # [D, 512]
        wkT = np.ascontiguousarray(wk.T).astype(bf16)
        wvT = np.ascontiguousarray(wv.T).astype(bf16)
        wpT = np.ascontiguousarray(
            W_proj[:, 512 * g : 512 * g + 512].T
        ).astype(bf16)  # [512, D]
        wslices.append((wqT, wkT, wvT, wpT))

    in_maps = []
    for c in range(NCORES):
        b, g = c // 2, c % 2
        xT = np.ascontiguousarray(x[b].T).astype(bf16)  # [D, T]
        wqT, wkT, wvT, wpT = wslices[g]
        in_maps.append(
            {
                "xT": xT,
                "wqT": wqT,
                "wkT": wkT,
                "wvT": wvT,
                "wpT": wpT,
                "cosr": cosr,
                "sinr": sinr,
                "esel": esel,
                "strictu": strictu,
            }
        )
    return in_maps


_NC_CACHE = {}


def _get_nc():
    if "nc" not in _NC_CACHE:
        _NC_CACHE["nc"] = _build()
    return _NC_CACHE["nc"]


def kernel(x, cos, sin, W_qkv, W_proj, _trace=False, _trace_cores=None):
    from concourse import bass_utils

    nc = _get_nc()
    in_maps = _host_inputs(x, cos, sin, W_qkv, W_proj)
    res = bass_utils.run_bass_kernel_spmd(
        nc,
        in_maps,
        core_ids=list(range(NCORES)),
        trace=_trace,
        trace_cores=_trace_cores,
    )
    out = np.zeros((B, T, D), np.float32)
    for c, r in enumerate(res.results):
        out[c // 2] += r["outp"]
    kernel.last_results = res
    return out
